# revision 11
# baseline (speedup 1.0000x reference)
"""Deformable conv block (offset conv -> bilinear sample -> conv -> BN -> ReLU)
on 8 Trainium2 NeuronCores, data-parallel over batch.

Self-contained: hardcodes all shapes. kernel(**inputs) takes full inputs,
shards batch across 8 cores, runs one Bass/Tile SPMD program, returns the
full [8, 64, 96, 96] float32 output.

v2 (bf16 pipeline):
  - gather table rows hold all 4 bilinear corners channel-interleaved in
    bf16 (512B rows) -> one 512B descriptor per (pixel, tap), 42.5 MB/core
  - one merged dma_gather per (chunk, tap): 1536 idxs (amortizes the
    ~1us per-instruction gpsimd descriptor-gen cost)
  - bilinear combine = broadcast mult + grouped 4->1 tensor_reduce (DVE,
    bf16, corner values adjacent so DVE perf modes stay on)
  - pixel-major -> channel-major pair transpose on the XBAR
    (dma_start_transpose) instead of PE matmul transposes
  - int16 gather indices built with wrap/doubling DMAs instead of PE
    double-transposes
  - offset conv + deformable conv matmuls in bf16 (fp32 PSUM accumulate)
  - offsets/weights/index math stays fp32 on DVE
  - BN stats AllReduce across 8 cores, scale/bias fold, ReLU, un-permute out
"""
import os
from contextlib import ExitStack

import numpy as np
import ml_dtypes

import concourse.bass as bass
import concourse.tile as tile
from concourse import bacc, mybir, bass_utils

dt = mybir.dt
AOT = mybir.AluOpType
AFT = mybir.ActivationFunctionType

# problem shapes
B, C, H, W, K = 8, 64, 96, 96, 3
HW = H * W                # 9216
K2 = K * K                # 9
NCORES = 8
EPS = 1e-5

# padded sample-grid geometry: padded coord = image coord + PADM
PADM = 3                  # margin for floor(py) in [0, 100]
PW = W + 2 * PADM + 1     # 103 padded grid width
PR = H + 2 * PADM + 1     # 103 padded grid rows
NQ = PW * PR              # 10609 rows in the 4-corner gather table
QCLAMP = float(W + 2 * PADM - 2)  # 100: floor clamp ceiling

NB = HW // 128            # 72 pixel-major block columns
CHUNK = 8                 # block columns per main-loop chunk (1024 idx = 32KB
NCH = NB // CHUNK         # per DMA engine: the single_packet gather limit)
NPIX = CHUNK * 128        # 1024 pixels per chunk
MAGIC = 8388608.0         # 2^23

_CACHE = {}


def _build(nc, ncores=NCORES, use_collective=True):
    STOP = os.environ.get("KSTOP", "full")
    xpad = nc.dram_tensor("xpad", [C, 98 * 98], dt.bfloat16, kind="ExternalInput").ap()
    x2pad = nc.dram_tensor("x2pad", [NQ, 256], dt.bfloat16, kind="ExternalInput").ap()
    woffT = nc.dram_tensor("woffT", [C, K2 * 18], dt.bfloat16, kind="ExternalInput").ap()
    wdefT = nc.dram_tensor("wdefT", [128, K2 * C], dt.bfloat16, kind="ExternalInput").ap()
    bpy = nc.dram_tensor("bpy", [128, K2 * NB], dt.float32, kind="ExternalInput").ap()
    bpx = nc.dram_tensor("bpx", [128, K2 * NB], dt.float32, kind="ExternalInput").ap()
    ident = nc.dram_tensor("ident", [128, 128], dt.float32, kind="ExternalInput").ap()
    bnc = nc.dram_tensor("bnc", [C, 2], dt.float32, kind="ExternalInput").ap()
    out_d = nc.dram_tensor("out", [C, HW], dt.float32, kind="ExternalOutput").ap()

    with tile.TileContext(nc) as tc:
        with ExitStack() as ctx:
            cpool = ctx.enter_context(tc.tile_pool(name="const", bufs=1))
            ppool = ctx.enter_context(tc.tile_pool(name="persist", bufs=1))
            spool = ctx.enter_context(tc.tile_pool(name="small", bufs=3))
            gpool = ctx.enter_context(tc.tile_pool(name="gather", bufs=5))
            tpool = ctx.enter_context(tc.tile_pool(name="tmul", bufs=4))
            wpool = ctx.enter_context(tc.tile_pool(name="work", bufs=3))
            dpool = ctx.enter_context(tc.tile_pool(name="dram", bufs=1, space="DRAM"))
            ps_m = ctx.enter_context(tc.tile_pool(name="ps_m", bufs=2, space="PSUM"))
            ps_o = ctx.enter_context(tc.tile_pool(name="ps_o", bufs=2, space="PSUM"))

            # ---- load constants ----
            woffT_s = cpool.tile([C, K2 * 18], dt.bfloat16)
            nc.sync.dma_start(woffT_s[:], woffT)
            wdefT_s = cpool.tile([128, K2 * C], dt.bfloat16)
            nc.sync.dma_start(wdefT_s[:], wdefT)
            bpy_s = cpool.tile([128, K2 * NB], dt.float32)
            nc.sync.dma_start(bpy_s[:], bpy)
            bpx_s = cpool.tile([128, K2 * NB], dt.float32)
            nc.sync.dma_start(bpx_s[:], bpx)
            id_s = cpool.tile([128, 128], dt.float32)
            nc.sync.dma_start(id_s[:], ident)
            bnc_s = cpool.tile([C, 2], dt.float32)
            nc.sync.dma_start(bnc_s[:], bnc)

            # ---- 1+2. offset conv (streamed) + transpose to pixel-major ----
            # offT_s[p, 18*b + j] = off[j, 128*b + p]
            offT_s = ppool.tile([128, NB * 18], dt.float32)
            xpv = xpad.rearrange("c (h w) -> c h w", w=98)
            for cc in range(24):            # chunks of 4 rows = 384 px = 3 blocks
                xpc = wpool.tile([C, 6 * 98], dt.bfloat16, tag="xpc")
                nc.sync.dma_start(
                    xpc[:].rearrange("c (h w) -> c h w", w=98),
                    xpv[:, 4 * cc: 4 * cc + 6, :])
                xv = xpc[:].rearrange("c (h w) -> c h w", w=98)
                po = ps_m.tile([18, 384], dt.float32, tag="ps_misc")
                for k in range(K2):
                    ky, kx = k // K, k % K
                    rhs = xv[:, ky: ky + 4, kx: kx + 96]
                    nc.tensor.matmul(po[:], woffT_s[:, 18 * k: 18 * k + 18], rhs,
                                     start=(k == 0), stop=(k == K2 - 1))
                offc = wpool.tile([18, 384], dt.float32, tag="offc")
                nc.scalar.copy(offc[:], po[:])
                for cb in range(3):
                    pt = ps_m.tile([128, 18], dt.float32, tag="ps_misc")
                    nc.tensor.transpose(pt[:], offc[:, 128 * cb: 128 * cb + 128],
                                        id_s[0:18, 0:18])
                    c = 3 * cc + cb
                    nc.vector.tensor_copy(offT_s[:, 18 * c: 18 * c + 18], pt[:])
            offT_v = offT_s[:].rearrange("p (b j) -> p b j", j=18)
            if STOP == "off":
                nc.sync.dma_start(out_d[:, 0:NB * 18].rearrange("c (a b) -> c a b", a=2),
                                  offT_s[:].rearrange("p (a b) -> p a b", a=2)[0:64])
                return

            # ---- 3. per tap: bilinear weights w4 (bf16) + int16 gather idx ----
            # w4b[k][p, b, j]: j in (c00, c10, c01, c11) matching table row order
            w4b_taps = []
            idx_taps = []
            for k in range(K2):
                idx_k = ppool.tile([128, HW // 16], dt.int16, tag=f"idx{k}",
                                   name=f"idx_tap{k}")
                idx_taps.append(idx_k)
            for k in range(K2):
                py = spool.tile([128, NB], dt.float32, tag="py")
                nc.vector.tensor_tensor(py[:], offT_v[:, :, 2 * k],
                                        bpy_s[:, NB * k: NB * k + NB], AOT.add)
                px = spool.tile([128, NB], dt.float32, tag="px")
                nc.vector.tensor_tensor(px[:], offT_v[:, :, 2 * k + 1],
                                        bpx_s[:, NB * k: NB * k + NB], AOT.add)
                ry = spool.tile([128, NB], dt.float32, tag="ry")
                nc.vector.tensor_scalar(ry[:], py[:], MAGIC - 0.5, None, AOT.add)
                fy = spool.tile([128, NB], dt.float32, tag="fy")
                nc.vector.tensor_scalar(fy[:], ry[:], MAGIC, None, AOT.subtract)
                rx = spool.tile([128, NB], dt.float32, tag="rx")
                nc.vector.tensor_scalar(rx[:], px[:], MAGIC - 0.5, None, AOT.add)
                fx = spool.tile([128, NB], dt.float32, tag="fx")
                nc.vector.tensor_scalar(fx[:], rx[:], MAGIC, None, AOT.subtract)
                ly = spool.tile([128, NB], dt.float32, tag="ly")
                nc.vector.tensor_tensor(ly[:], py[:], fy[:], AOT.subtract)
                lx = spool.tile([128, NB], dt.float32, tag="lx")
                nc.vector.tensor_tensor(lx[:], px[:], fx[:], AOT.subtract)
                wy0 = spool.tile([128, NB], dt.float32, tag="wy0")
                nc.vector.tensor_scalar(wy0[:], ly[:], -1.0, 1.0, AOT.mult, AOT.add)
                wx0 = spool.tile([128, NB], dt.float32, tag="wx0")
                nc.vector.tensor_scalar(wx0[:], lx[:], -1.0, 1.0, AOT.mult, AOT.add)
                w4f = spool.tile([128, NB * 4], dt.float32, tag="w4f")
                w4fv = w4f[:].rearrange("p (b j) -> p b j", j=4)
                nc.vector.tensor_tensor(w4fv[:, :, 0], wy0[:], wx0[:], AOT.mult)
                nc.vector.tensor_tensor(w4fv[:, :, 1], ly[:], wx0[:], AOT.mult)
                nc.vector.tensor_tensor(w4fv[:, :, 2], wy0[:], lx[:], AOT.mult)
                nc.vector.tensor_tensor(w4fv[:, :, 3], ly[:], lx[:], AOT.mult)
                w4b = ppool.tile([128, NB * 4], dt.bfloat16, tag=f"w4b{k}")
                nc.vector.tensor_copy(w4b[:], w4f[:])
                w4b_taps.append(w4b)
                # gather row index q = qy * PW + qx (exact small integers)
                qy = spool.tile([128, NB], dt.float32, tag="qy")
                nc.vector.tensor_scalar(qy[:], fy[:], 0.0, QCLAMP, AOT.max, AOT.min)
                qx = spool.tile([128, NB], dt.float32, tag="qx")
                nc.vector.tensor_scalar(qx[:], fx[:], 0.0, QCLAMP, AOT.max, AOT.min)
                qf = spool.tile([128, NB], dt.float32, tag="qf")
                nc.vector.scalar_tensor_tensor(qf[:], qy[:], float(PW), qx[:],
                                               AOT.mult, AOT.add)
                # idx 16-row-wrap via PE double-transpose (T1 then 8x T2)
                t1p = ps_m.tile([NB, 128], dt.float32, tag="ps_misc")
                nc.tensor.transpose(t1p[:], qf[:], id_s[:])
                t1s = spool.tile([NB, 128], dt.float32, tag="t1s")
                nc.vector.tensor_copy(t1s[:], t1p[:])
                idx_k = idx_taps[k]
                stv = idx_k[0:16, :].rearrange("p (b r) -> p b r", r=8)
                for r in range(8):
                    t2p = ps_m.tile([16, NB], dt.float32, tag="ps_misc")
                    nc.tensor.transpose(t2p[:], t1s[:, 16 * r: 16 * r + 16],
                                        id_s[0:NB, 0:NB])
                    nc.any.tensor_copy(stv[:, :, r], t2p[:])
                # ---- 4. replicate idx rows 0..15 to all 8 16-row groups ----
                nc.sync.dma_start(idx_k[16:32, :], idx_k[0:16, :])
                nc.sync.dma_start(idx_k[32:64, :], idx_k[0:32, :])
                nc.sync.dma_start(idx_k[64:128, :], idx_k[0:64, :])

            if STOP == "idx":
                nc.sync.dma_start(
                    out_d[0:64, 0:HW // 16].bitcast(dt.int16)[:, 0:HW // 16],
                    idx_taps[0][0:64, :])
                return

            # ---- 5. main loop: gather -> weight+reduce -> xbarT -> conv ----
            x2win = bass.AP(x2pad.tensor, 0, [[256, NQ], [1, 256]])
            conv_s = ppool.tile([C, HW], dt.float32)
            sums = ppool.tile([C, 12], dt.float32)
            sqs = ppool.tile([C, 12], dt.float32)
            NCH_RUN = 1 if STOP in ("g1", "wsum", "tr") else NCH
            NIC = NPIX // 16          # idx cols per (chunk, tap): 64
            for ch in range(NCH_RUN):
                # 2 bank-aligned accumulation groups: even | odd blocks
                po = ps_o.tile([C, 1024], dt.float32, tag="ps_out")
                for k in range(K2):
                    g_t = gpool.tile([128, CHUNK * 256], dt.bfloat16, tag="g")
                    nc.gpsimd.dma_gather(
                        out_ap=g_t[:].rearrange("p (b e) -> p b e", e=256),
                        in_ap=x2win,
                        idxs_ap=idx_taps[k][:, NIC * ch: NIC * ch + NIC],
                        num_idxs=NPIX,
                        num_idxs_reg=NPIX,
                        elem_size=256,
                        elem_step=256,
                    )
                    if STOP == "g1":
                        nc.sync.dma_start(
                            out_d[0:64, 0:CHUNK * 128].bitcast(dt.bfloat16)[:, 0:CHUNK * 256],
                            g_t[0:64, :])
                        return
                    # bilinear combine: t = g * w4 (broadcast over channels),
                    # then reduce over the 4 corners (innermost)
                    gv4 = g_t[:].rearrange("p (b c j) -> p b c j", c=C, j=4)
                    w4v = w4b_taps[k][:].rearrange("p (b u j) -> p b u j", u=1, j=4)
                    wj = w4v[:, CHUNK * ch: CHUNK * ch + CHUNK]
                    a1, a2 = bass.broadcast_tensor_aps(gv4, wj)
                    t_t = tpool.tile([128, CHUNK * 256], dt.bfloat16, tag="t")
                    nc.vector.tensor_tensor(
                        t_t[:].rearrange("p (b c j) -> p b c j", c=C, j=4), a1, a2,
                        AOT.mult)
                    # 4-corner sum as two adds: pairs (2x-mode) then final
                    tv = t_t[:].rearrange("p (bc j) -> p bc j", j=4)
                    s2 = wpool.tile([128, CHUNK * C * 2], dt.bfloat16, tag="s2",
                                    bufs=6)
                    s2v = s2[:].rearrange("p (bc j) -> p bc j", j=2)
                    nc.vector.tensor_tensor(s2v, tv[:, :, 0:2], tv[:, :, 2:4],
                                            AOT.add)
                    s_t = wpool.tile([128, CHUNK * C], dt.bfloat16, tag="s",
                                     bufs=10)
                    nc.vector.tensor_tensor(s_t[:], s2v[:, :, 0], s2v[:, :, 1],
                                            AOT.add)
                    if STOP == "wsum":
                        if k == K2 - 1:
                            nc.sync.dma_start(
                                out_d[0:64, 0:CHUNK * 32].bitcast(dt.bfloat16)[:, 0:CHUNK * C],
                                s_t[0:64, :])
                            return
                        continue
                    # pair transpose -> channel-major sampled (XBAR dma)
                    samp = wpool.tile([128, CHUNK * C], dt.bfloat16, tag="samp",
                                      bufs=10)
                    nc.sync.dma_start_transpose(
                        samp[:].rearrange("p (bb q) -> p bb q", q=128), s_t[:])
                    if STOP == "tr":
                        if k == K2 - 1:
                            nc.sync.dma_start(
                                out_d[0:64, 0:CHUNK * 32].bitcast(dt.bfloat16)[:, 0:CHUNK * C],
                                samp[0:64, :])
                            return
                        continue
                    # conv matmuls: accumulate over taps, parity-major out cols
                    st, sp = (k == 0), (k == K2 - 1)
                    lhe = wdefT_s[0:64, C * k: C * k + C]
                    lho = wdefT_s[64:128, C * k: C * k + C]
                    nc.tensor.matmul(po[:, 0:512], lhe, samp[0:64, :],
                                     start=st, stop=sp)
                    nc.tensor.matmul(po[:, 512:1024], lho, samp[64:128, :],
                                     start=st, stop=sp)
                # copy conv chunk to SBUF + per-chunk sum / sumsq
                cview = conv_s[:, NPIX * ch: NPIX * ch + NPIX]
                nc.scalar.activation(cview, po[:], AFT.Copy,
                                     accum_out=sums[:, ch: ch + 1])
                scr = wpool.tile([C, NPIX], dt.float32, tag="scr", bufs=1)
                nc.scalar.activation(scr[:], cview, AFT.Square,
                                     accum_out=sqs[:, ch: ch + 1])

            if STOP == "loop":
                nc.sync.dma_start(out_d[:], conv_s[:])
                return

            # ---- 6. BN stats allreduce + normalize + relu + output ----
            st2 = ppool.tile([C, 2], dt.float32)
            nc.vector.tensor_reduce(st2[:, 0:1], sums[:, 0:NCH],
                                    mybir.AxisListType.X, AOT.add)
            nc.vector.tensor_reduce(st2[:, 1:2], sqs[:, 0:NCH],
                                    mybir.AxisListType.X, AOT.add)
            bi = dpool.tile([C, 2], dt.float32)
            bo = dpool.tile([C, 2], dt.float32)
            nc.sync.dma_start(bi[:], st2[:])
            if use_collective:
                nc.gpsimd.collective_compute(
                    "AllReduce", AOT.add,
                    replica_groups=[list(range(ncores))],
                    ins=[bi.opt()], outs=[bo.opt()])
            else:
                nc.sync.dma_start(bo[:], bi[:])
            ast = ppool.tile([C, 2], dt.float32)
            nc.sync.dma_start(ast[:], bo[:])

            inv_n = 1.0 / float(ncores * HW)
            mean = ppool.tile([C, 1], dt.float32)
            nc.vector.tensor_scalar(mean[:], ast[:, 0:1], inv_n, None, AOT.mult)
            msq = ppool.tile([C, 1], dt.float32)
            nc.vector.tensor_scalar(msq[:], ast[:, 1:2], inv_n, None, AOT.mult)
            m2 = ppool.tile([C, 1], dt.float32)
            nc.vector.tensor_tensor(m2[:], mean[:], mean[:], AOT.mult)
            var = ppool.tile([C, 1], dt.float32)
            nc.vector.tensor_tensor(var[:], msq[:], m2[:], AOT.subtract)
            vare = ppool.tile([C, 1], dt.float32)
            nc.vector.tensor_scalar(vare[:], var[:], EPS, None, AOT.add)
            sd = ppool.tile([C, 1], dt.float32)
            nc.scalar.activation(sd[:], vare[:], AFT.Sqrt)
            inv = ppool.tile([C, 1], dt.float32)
            nc.vector.reciprocal(inv[:], sd[:])
            scl = ppool.tile([C, 1], dt.float32)
            nc.vector.tensor_tensor(scl[:], bnc_s[:, 0:1], inv[:], AOT.mult)
            mt = ppool.tile([C, 1], dt.float32)
            nc.vector.tensor_tensor(mt[:], mean[:], scl[:], AOT.mult)
            bia = ppool.tile([C, 1], dt.float32)
            nc.vector.tensor_tensor(bia[:], bnc_s[:, 1:2], mt[:], AOT.subtract)

            ov = out_d.rearrange("c (n q) -> c n q", q=128)
            for ch in range(NCH):
                on = wpool.tile([C, NPIX], dt.float32, tag="on")
                nc.scalar.activation(on[:], conv_s[:, NPIX * ch: NPIX * ch + NPIX],
                                     AFT.Relu, bias=bia[:], scale=scl[:])
                onv = on[:].rearrange("c (n q) -> c n q", q=128)
                # even local blocks -> even global block slots, odd -> odd
                nc.sync.dma_start(ov[:, CHUNK * ch: CHUNK * ch + CHUNK: 2, :],
                                  onv[:, 0:CHUNK // 2, :])
                nc.sync.dma_start(ov[:, CHUNK * ch + 1: CHUNK * ch + CHUNK: 2, :],
                                  onv[:, CHUNK // 2:CHUNK, :])


def _prep_core(xb, w_off, b_off, w_def, gamma, beta):
    """Host-side input prep for one batch item. xb: [64, 96, 96] f32."""
    bf16 = ml_dtypes.bfloat16
    ins = {}
    # xpad: zero-pad by 1 for the 3x3 offset conv (bf16)
    xp = np.zeros((C, 98, 98), np.float32)
    xp[:, 1:97, 1:97] = xb
    ins["xpad"] = xp.reshape(C, 98 * 98).astype(bf16)
    # x2pad: 4-corner gather table, channel-interleaved bf16.
    # row q = y0*PW + x0 holds [xz[y0,x0,c], xz[y0+1,x0,c], xz[y0,x0+1,c],
    # xz[y0+1,x0+1,c]] for c in 0..63 -> 256 bf16 = 512B
    xz = np.zeros((PR + 2, PW, C), np.float32)
    xz[PADM:PADM + H, PADM:PADM + W] = xb.transpose(1, 2, 0)
    xzf = xz.reshape((PR + 2) * PW, C)
    tab = np.stack([xzf[0:NQ], xzf[PW:NQ + PW], xzf[1:NQ + 1],
                    xzf[PW + 1:NQ + PW + 1]], axis=2)  # [NQ, C, 4]
    ins["x2pad"] = tab.reshape(NQ, 4 * C).astype(bf16)
    # weight rearrangements
    wofft = np.zeros((C, K2 * 18), np.float32)
    for k in range(K2):
        wofft[:, 18 * k:18 * k + 18] = w_off[:, :, k // K, k % K].T
    ins["woffT"] = wofft.astype(bf16)
    wdeft = np.zeros((128, K2 * C), np.float32)
    for k in range(K2):
        blk = w_def[:, :, k // K, k % K].T  # [cin, cout]
        wdeft[0:64, C * k:C * k + C] = blk
        wdeft[64:128, C * k:C * k + C] = blk
    ins["wdefT"] = wdeft.astype(bf16)
    # base grids (pixel-major [128, 72] per tap), fold b_off and pad margin
    pixi = np.arange(HW, dtype=np.int64)
    ygrid = (pixi // W).astype(np.float32)
    xgrid = (pixi % W).astype(np.float32)
    ypm = ygrid.reshape(NB, 128).T    # [p, b] pixel-major
    xpm = xgrid.reshape(NB, 128).T
    bpy = np.zeros((128, K2 * NB), np.float32)
    bpx = np.zeros((128, K2 * NB), np.float32)
    for k in range(K2):
        ky, kx = k // K - 1, k % K - 1
        bpy[:, NB * k:NB * k + NB] = ypm + (ky + PADM + b_off[2 * k])
        bpx[:, NB * k:NB * k + NB] = xpm + (kx + PADM + b_off[2 * k + 1])
    ins["bpy"] = bpy
    ins["bpx"] = bpx
    ins["ident"] = np.eye(128, dtype=np.float32)
    ins["bnc"] = np.stack([gamma, beta], axis=1).astype(np.float32)
    return ins


def _get_nc():
    if "nc" not in _CACHE:
        nc = bacc.Bacc("TRN2", target_bir_lowering=False, debug=False,
                       num_devices=NCORES)
        _build(nc)
        nc.compile()
        _CACHE["nc"] = nc
    return _CACHE["nc"]


def kernel(x, w_off, b_off, w_def, b_def, gamma, beta, trace=False, tmpdir=None):
    x = np.asarray(x, np.float32)
    w_off = np.asarray(w_off, np.float32)
    b_off = np.asarray(b_off, np.float32)
    w_def = np.asarray(w_def, np.float32)
    gamma = np.asarray(gamma, np.float32)
    beta = np.asarray(beta, np.float32)
    # b_def cancels exactly in training-mode BN; accepted but unused.
    nc = _get_nc()
    in_maps = [_prep_core(x[b], w_off, b_off, w_def, gamma, beta)
               for b in range(B)]
    res = bass_utils.run_bass_kernel_spmd(
        nc, in_maps, core_ids=list(range(NCORES)), trace=trace, tmpdir=tmpdir)
    out = np.stack([res.results[b]["out"].reshape(C, H, W) for b in range(B)])
    if trace:
        kernel.last_exec_time_ns = res.exec_time_ns
        kernel.last_results = res
    return out



# revision 13
# speedup vs baseline: 1.0071x; 1.0071x over previous
"""Deformable conv block (offset conv -> bilinear sample -> conv -> BN -> ReLU)
on 8 Trainium2 NeuronCores, data-parallel over batch.

Self-contained: hardcodes all shapes. kernel(**inputs) takes full inputs,
shards batch across 8 cores, runs one Bass/Tile SPMD program, returns the
full [8, 64, 96, 96] float32 output.

v2 (bf16 pipeline):
  - gather table rows hold all 4 bilinear corners channel-interleaved in
    bf16 (512B rows) -> one 512B descriptor per (pixel, tap), 42.5 MB/core
  - one merged dma_gather per (chunk, tap): 1536 idxs (amortizes the
    ~1us per-instruction gpsimd descriptor-gen cost)
  - bilinear combine = broadcast mult + grouped 4->1 tensor_reduce (DVE,
    bf16, corner values adjacent so DVE perf modes stay on)
  - pixel-major -> channel-major pair transpose on the XBAR
    (dma_start_transpose) instead of PE matmul transposes
  - int16 gather indices built with wrap/doubling DMAs instead of PE
    double-transposes
  - offset conv + deformable conv matmuls in bf16 (fp32 PSUM accumulate)
  - offsets/weights/index math stays fp32 on DVE
  - BN stats AllReduce across 8 cores, scale/bias fold, ReLU, un-permute out
"""
import os
from contextlib import ExitStack

import numpy as np
import ml_dtypes

import concourse.bass as bass
import concourse.tile as tile
from concourse import bacc, mybir, bass_utils

dt = mybir.dt
AOT = mybir.AluOpType
AFT = mybir.ActivationFunctionType

# problem shapes
B, C, H, W, K = 8, 64, 96, 96, 3
HW = H * W                # 9216
K2 = K * K                # 9
NCORES = 8
EPS = 1e-5

# padded sample-grid geometry: padded coord = image coord + PADM
PADM = 3                  # margin for floor(py) in [0, 100]
PW = W + 2 * PADM + 1     # 103 padded grid width
PR = H + 2 * PADM + 1     # 103 padded grid rows
NQ = PW * PR              # 10609 rows in the 4-corner gather table
QCLAMP = float(W + 2 * PADM - 2)  # 100: floor clamp ceiling

NB = HW // 128            # 72 pixel-major block columns
CHUNK = 8                 # block columns per main-loop chunk (1024 idx = 32KB
NCH = NB // CHUNK         # per DMA engine: the single_packet gather limit)
NPIX = CHUNK * 128        # 1024 pixels per chunk
MAGIC = 8388608.0         # 2^23

_CACHE = {}


def _build(nc, ncores=NCORES, use_collective=True):
    STOP = os.environ.get("KSTOP", "full")
    xpad = nc.dram_tensor("xpad", [C, 98 * 98], dt.bfloat16, kind="ExternalInput").ap()
    x2pad = nc.dram_tensor("x2pad", [NQ, 256], dt.bfloat16, kind="ExternalInput").ap()
    woffT = nc.dram_tensor("woffT", [C, K2 * 18], dt.bfloat16, kind="ExternalInput").ap()
    wdefT = nc.dram_tensor("wdefT", [128, K2 * C], dt.bfloat16, kind="ExternalInput").ap()
    bpy = nc.dram_tensor("bpy", [128, K2 * NB], dt.float32, kind="ExternalInput").ap()
    bpx = nc.dram_tensor("bpx", [128, K2 * NB], dt.float32, kind="ExternalInput").ap()
    ident = nc.dram_tensor("ident", [128, 128], dt.float32, kind="ExternalInput").ap()
    bnc = nc.dram_tensor("bnc", [C, 2], dt.float32, kind="ExternalInput").ap()
    out_d = nc.dram_tensor("out", [C, HW], dt.float32, kind="ExternalOutput").ap()

    with tile.TileContext(nc) as tc:
        with ExitStack() as ctx:
            cpool = ctx.enter_context(tc.tile_pool(name="const", bufs=1))
            ppool = ctx.enter_context(tc.tile_pool(name="persist", bufs=1))
            spool = ctx.enter_context(tc.tile_pool(name="small", bufs=3))
            gpool = ctx.enter_context(tc.tile_pool(name="gather", bufs=5))
            tpool = ctx.enter_context(tc.tile_pool(name="tmul", bufs=4))
            wpool = ctx.enter_context(tc.tile_pool(name="work", bufs=3))
            dpool = ctx.enter_context(tc.tile_pool(name="dram", bufs=1, space="DRAM"))
            ps_m = ctx.enter_context(tc.tile_pool(name="ps_m", bufs=2, space="PSUM"))
            ps_o = ctx.enter_context(tc.tile_pool(name="ps_o", bufs=2, space="PSUM"))

            # ---- load constants ----
            woffT_s = cpool.tile([C, K2 * 18], dt.bfloat16)
            nc.sync.dma_start(woffT_s[:], woffT)
            wdefT_s = cpool.tile([128, K2 * C], dt.bfloat16)
            nc.sync.dma_start(wdefT_s[:], wdefT)
            bpy_s = cpool.tile([128, K2 * NB], dt.float32)
            nc.sync.dma_start(bpy_s[:], bpy)
            bpx_s = cpool.tile([128, K2 * NB], dt.float32)
            nc.sync.dma_start(bpx_s[:], bpx)
            id_s = cpool.tile([128, 128], dt.float32)
            nc.sync.dma_start(id_s[:], ident)
            bnc_s = cpool.tile([C, 2], dt.float32)
            nc.sync.dma_start(bnc_s[:], bnc)

            # ---- 1+2. offset conv (streamed) + transpose to pixel-major ----
            # offT_s[p, 18*b + j] = off[j, 128*b + p]
            offT_s = ppool.tile([128, NB * 18], dt.float32)
            xpv = xpad.rearrange("c (h w) -> c h w", w=98)
            for cc in range(24):            # chunks of 4 rows = 384 px = 3 blocks
                xpc = wpool.tile([C, 6 * 98], dt.bfloat16, tag="xpc")
                nc.sync.dma_start(
                    xpc[:].rearrange("c (h w) -> c h w", w=98),
                    xpv[:, 4 * cc: 4 * cc + 6, :])
                xv = xpc[:].rearrange("c (h w) -> c h w", w=98)
                po = ps_m.tile([18, 384], dt.float32, tag="ps_misc")
                for k in range(K2):
                    ky, kx = k // K, k % K
                    rhs = xv[:, ky: ky + 4, kx: kx + 96]
                    nc.tensor.matmul(po[:], woffT_s[:, 18 * k: 18 * k + 18], rhs,
                                     start=(k == 0), stop=(k == K2 - 1))
                offc = wpool.tile([18, 384], dt.float32, tag="offc")
                nc.scalar.copy(offc[:], po[:])
                for cb in range(3):
                    pt = ps_m.tile([128, 18], dt.float32, tag="ps_misc")
                    nc.tensor.transpose(pt[:], offc[:, 128 * cb: 128 * cb + 128],
                                        id_s[0:18, 0:18])
                    c = 3 * cc + cb
                    nc.vector.tensor_copy(offT_s[:, 18 * c: 18 * c + 18], pt[:])
            offT_v = offT_s[:].rearrange("p (b j) -> p b j", j=18)
            if STOP == "off":
                nc.sync.dma_start(out_d[:, 0:NB * 18].rearrange("c (a b) -> c a b", a=2),
                                  offT_s[:].rearrange("p (a b) -> p a b", a=2)[0:64])
                return

            # ---- 3. per tap: bilinear weights w4 (bf16) + int16 gather idx ----
            # w4b[k][p, b, j]: j in (c00, c10, c01, c11) matching table row order
            w4b_taps = []
            idx_taps = []
            for k in range(K2):
                idx_k = ppool.tile([128, HW // 16], dt.int16, tag=f"idx{k}",
                                   name=f"idx_tap{k}")
                idx_taps.append(idx_k)
            for k in range(K2):
                py = spool.tile([128, NB], dt.float32, tag="py")
                nc.vector.tensor_tensor(py[:], offT_v[:, :, 2 * k],
                                        bpy_s[:, NB * k: NB * k + NB], AOT.add)
                px = spool.tile([128, NB], dt.float32, tag="px")
                nc.vector.tensor_tensor(px[:], offT_v[:, :, 2 * k + 1],
                                        bpx_s[:, NB * k: NB * k + NB], AOT.add)
                ry = spool.tile([128, NB], dt.float32, tag="ry")
                nc.vector.tensor_scalar(ry[:], py[:], MAGIC - 0.5, None, AOT.add)
                fy = spool.tile([128, NB], dt.float32, tag="fy")
                nc.vector.tensor_scalar(fy[:], ry[:], MAGIC, None, AOT.subtract)
                rx = spool.tile([128, NB], dt.float32, tag="rx")
                nc.vector.tensor_scalar(rx[:], px[:], MAGIC - 0.5, None, AOT.add)
                fx = spool.tile([128, NB], dt.float32, tag="fx")
                nc.vector.tensor_scalar(fx[:], rx[:], MAGIC, None, AOT.subtract)
                ly = spool.tile([128, NB], dt.float32, tag="ly")
                nc.vector.tensor_tensor(ly[:], py[:], fy[:], AOT.subtract)
                lx = spool.tile([128, NB], dt.float32, tag="lx")
                nc.vector.tensor_tensor(lx[:], px[:], fx[:], AOT.subtract)
                wy0 = spool.tile([128, NB], dt.float32, tag="wy0")
                nc.vector.tensor_scalar(wy0[:], ly[:], -1.0, 1.0, AOT.mult, AOT.add)
                wx0 = spool.tile([128, NB], dt.float32, tag="wx0")
                nc.vector.tensor_scalar(wx0[:], lx[:], -1.0, 1.0, AOT.mult, AOT.add)
                w4f = spool.tile([128, NB * 4], dt.float32, tag="w4f")
                w4fv = w4f[:].rearrange("p (b j) -> p b j", j=4)
                nc.vector.tensor_tensor(w4fv[:, :, 0], wy0[:], wx0[:], AOT.mult)
                nc.vector.tensor_tensor(w4fv[:, :, 1], ly[:], wx0[:], AOT.mult)
                nc.vector.tensor_tensor(w4fv[:, :, 2], wy0[:], lx[:], AOT.mult)
                nc.vector.tensor_tensor(w4fv[:, :, 3], ly[:], lx[:], AOT.mult)
                w4b = ppool.tile([128, NB * 4], dt.bfloat16, tag=f"w4b{k}")
                nc.vector.tensor_copy(w4b[:], w4f[:])
                w4b_taps.append(w4b)
                # gather row index q = qy * PW + qx (exact small integers)
                qy = spool.tile([128, NB], dt.float32, tag="qy")
                nc.vector.tensor_scalar(qy[:], fy[:], 0.0, QCLAMP, AOT.max, AOT.min)
                qx = spool.tile([128, NB], dt.float32, tag="qx")
                nc.vector.tensor_scalar(qx[:], fx[:], 0.0, QCLAMP, AOT.max, AOT.min)
                qf = spool.tile([128, NB], dt.float32, tag="qf")
                nc.vector.scalar_tensor_tensor(qf[:], qy[:], float(PW), qx[:],
                                               AOT.mult, AOT.add)
                # idx 16-row-wrap via PE double-transpose (T1 then 8x T2)
                t1p = ps_m.tile([NB, 128], dt.float32, tag="ps_misc")
                nc.tensor.transpose(t1p[:], qf[:], id_s[:])
                t1s = spool.tile([NB, 128], dt.float32, tag="t1s")
                nc.vector.tensor_copy(t1s[:], t1p[:])
                idx_k = idx_taps[k]
                stv = idx_k[0:16, :].rearrange("p (b r) -> p b r", r=8)
                for r in range(8):
                    t2p = ps_m.tile([16, NB], dt.float32, tag="ps_misc")
                    nc.tensor.transpose(t2p[:], t1s[:, 16 * r: 16 * r + 16],
                                        id_s[0:NB, 0:NB])
                    nc.any.tensor_copy(stv[:, :, r], t2p[:])
                # ---- 4. replicate idx rows 0..15 to all 8 16-row groups ----
                nc.sync.dma_start(idx_k[16:32, :], idx_k[0:16, :])
                nc.sync.dma_start(idx_k[32:64, :], idx_k[0:32, :])
                nc.sync.dma_start(idx_k[64:128, :], idx_k[0:64, :])

            if STOP == "idx":
                nc.sync.dma_start(
                    out_d[0:64, 0:HW // 16].bitcast(dt.int16)[:, 0:HW // 16],
                    idx_taps[0][0:64, :])
                return

            # ---- 5. main loop: gather -> weight+reduce -> xbarT -> conv ----
            x2win = bass.AP(x2pad.tensor, 0, [[256, NQ], [1, 256]])
            conv_s = ppool.tile([C, HW], dt.float32)
            sums = ppool.tile([C, 12], dt.float32)
            sqs = ppool.tile([C, 12], dt.float32)
            NCH_RUN = 1 if STOP in ("g1", "wsum", "tr") else NCH
            NIC = NPIX // 16          # idx cols per (chunk, tap): 64
            for ch in range(NCH_RUN):
                # 2 bank-aligned accumulation groups: even | odd blocks
                po = ps_o.tile([C, 1024], dt.float32, tag="ps_out")
                for k in range(K2):
                    g_t = gpool.tile([128, CHUNK * 256], dt.bfloat16, tag="g")
                    nc.gpsimd.dma_gather(
                        out_ap=g_t[:].rearrange("p (b e) -> p b e", e=256),
                        in_ap=x2win,
                        idxs_ap=idx_taps[k][:, NIC * ch: NIC * ch + NIC],
                        num_idxs=NPIX,
                        num_idxs_reg=NPIX,
                        elem_size=256,
                        elem_step=256,
                        queue_num=1,
                    )
                    if STOP == "g1":
                        nc.sync.dma_start(
                            out_d[0:64, 0:CHUNK * 128].bitcast(dt.bfloat16)[:, 0:CHUNK * 256],
                            g_t[0:64, :])
                        return
                    # bilinear combine: t = g * w4 (broadcast over channels),
                    # then reduce over the 4 corners (innermost)
                    gv4 = g_t[:].rearrange("p (b c j) -> p b c j", c=C, j=4)
                    w4v = w4b_taps[k][:].rearrange("p (b u j) -> p b u j", u=1, j=4)
                    wj = w4v[:, CHUNK * ch: CHUNK * ch + CHUNK]
                    a1, a2 = bass.broadcast_tensor_aps(gv4, wj)
                    t_t = tpool.tile([128, CHUNK * 256], dt.bfloat16, tag="t")
                    nc.vector.tensor_tensor(
                        t_t[:].rearrange("p (b c j) -> p b c j", c=C, j=4), a1, a2,
                        AOT.mult)
                    # 4-corner sum as two adds: pairs (2x-mode) then final
                    tv = t_t[:].rearrange("p (bc j) -> p bc j", j=4)
                    s2 = wpool.tile([128, CHUNK * C * 2], dt.bfloat16, tag="s2",
                                    bufs=6)
                    s2v = s2[:].rearrange("p (bc j) -> p bc j", j=2)
                    nc.vector.tensor_tensor(s2v, tv[:, :, 0:2], tv[:, :, 2:4],
                                            AOT.add)
                    s_t = wpool.tile([128, CHUNK * C], dt.bfloat16, tag="s",
                                     bufs=10)
                    nc.vector.tensor_tensor(s_t[:], s2v[:, :, 0], s2v[:, :, 1],
                                            AOT.add)
                    if STOP == "wsum":
                        if k == K2 - 1:
                            nc.sync.dma_start(
                                out_d[0:64, 0:CHUNK * 32].bitcast(dt.bfloat16)[:, 0:CHUNK * C],
                                s_t[0:64, :])
                            return
                        continue
                    # pair transpose -> channel-major sampled (XBAR dma)
                    samp = wpool.tile([128, CHUNK * C], dt.bfloat16, tag="samp",
                                      bufs=10)
                    nc.sync.dma_start_transpose(
                        samp[:].rearrange("p (bb q) -> p bb q", q=128), s_t[:])
                    if STOP == "tr":
                        if k == K2 - 1:
                            nc.sync.dma_start(
                                out_d[0:64, 0:CHUNK * 32].bitcast(dt.bfloat16)[:, 0:CHUNK * C],
                                samp[0:64, :])
                            return
                        continue
                    # conv matmuls: accumulate over taps, parity-major out cols
                    st, sp = (k == 0), (k == K2 - 1)
                    lhe = wdefT_s[0:64, C * k: C * k + C]
                    lho = wdefT_s[64:128, C * k: C * k + C]
                    nc.tensor.matmul(po[:, 0:512], lhe, samp[0:64, :],
                                     start=st, stop=sp)
                    nc.tensor.matmul(po[:, 512:1024], lho, samp[64:128, :],
                                     start=st, stop=sp)
                # copy conv chunk to SBUF + per-chunk sum / sumsq
                cview = conv_s[:, NPIX * ch: NPIX * ch + NPIX]
                nc.scalar.activation(cview, po[:], AFT.Copy,
                                     accum_out=sums[:, ch: ch + 1])
                scr = wpool.tile([C, NPIX], dt.float32, tag="scr", bufs=1)
                nc.scalar.activation(scr[:], cview, AFT.Square,
                                     accum_out=sqs[:, ch: ch + 1])

            if STOP == "loop":
                nc.sync.dma_start(out_d[:], conv_s[:])
                return

            # ---- 6. BN stats allreduce + normalize + relu + output ----
            st2 = ppool.tile([C, 2], dt.float32)
            nc.vector.tensor_reduce(st2[:, 0:1], sums[:, 0:NCH],
                                    mybir.AxisListType.X, AOT.add)
            nc.vector.tensor_reduce(st2[:, 1:2], sqs[:, 0:NCH],
                                    mybir.AxisListType.X, AOT.add)
            bi = dpool.tile([C, 2], dt.float32)
            bo = dpool.tile([C, 2], dt.float32)
            nc.sync.dma_start(bi[:], st2[:])
            if use_collective:
                nc.gpsimd.collective_compute(
                    "AllReduce", AOT.add,
                    replica_groups=[list(range(ncores))],
                    ins=[bi.opt()], outs=[bo.opt()])
            else:
                nc.sync.dma_start(bo[:], bi[:])
            ast = ppool.tile([C, 2], dt.float32)
            nc.sync.dma_start(ast[:], bo[:])

            inv_n = 1.0 / float(ncores * HW)
            mean = ppool.tile([C, 1], dt.float32)
            nc.vector.tensor_scalar(mean[:], ast[:, 0:1], inv_n, None, AOT.mult)
            msq = ppool.tile([C, 1], dt.float32)
            nc.vector.tensor_scalar(msq[:], ast[:, 1:2], inv_n, None, AOT.mult)
            m2 = ppool.tile([C, 1], dt.float32)
            nc.vector.tensor_tensor(m2[:], mean[:], mean[:], AOT.mult)
            var = ppool.tile([C, 1], dt.float32)
            nc.vector.tensor_tensor(var[:], msq[:], m2[:], AOT.subtract)
            vare = ppool.tile([C, 1], dt.float32)
            nc.vector.tensor_scalar(vare[:], var[:], EPS, None, AOT.add)
            sd = ppool.tile([C, 1], dt.float32)
            nc.scalar.activation(sd[:], vare[:], AFT.Sqrt)
            inv = ppool.tile([C, 1], dt.float32)
            nc.vector.reciprocal(inv[:], sd[:])
            scl = ppool.tile([C, 1], dt.float32)
            nc.vector.tensor_tensor(scl[:], bnc_s[:, 0:1], inv[:], AOT.mult)
            mt = ppool.tile([C, 1], dt.float32)
            nc.vector.tensor_tensor(mt[:], mean[:], scl[:], AOT.mult)
            bia = ppool.tile([C, 1], dt.float32)
            nc.vector.tensor_tensor(bia[:], bnc_s[:, 1:2], mt[:], AOT.subtract)

            ov = out_d.rearrange("c (n q) -> c n q", q=128)
            for ch in range(NCH):
                on = wpool.tile([C, NPIX], dt.float32, tag="on")
                nc.scalar.activation(on[:], conv_s[:, NPIX * ch: NPIX * ch + NPIX],
                                     AFT.Relu, bias=bia[:], scale=scl[:])
                onv = on[:].rearrange("c (n q) -> c n q", q=128)
                # even local blocks -> even global block slots, odd -> odd
                nc.sync.dma_start(ov[:, CHUNK * ch: CHUNK * ch + CHUNK: 2, :],
                                  onv[:, 0:CHUNK // 2, :])
                nc.sync.dma_start(ov[:, CHUNK * ch + 1: CHUNK * ch + CHUNK: 2, :],
                                  onv[:, CHUNK // 2:CHUNK, :])


def _prep_core(xb, w_off, b_off, w_def, gamma, beta):
    """Host-side input prep for one batch item. xb: [64, 96, 96] f32."""
    bf16 = ml_dtypes.bfloat16
    ins = {}
    # xpad: zero-pad by 1 for the 3x3 offset conv (bf16)
    xp = np.zeros((C, 98, 98), np.float32)
    xp[:, 1:97, 1:97] = xb
    ins["xpad"] = xp.reshape(C, 98 * 98).astype(bf16)
    # x2pad: 4-corner gather table, channel-interleaved bf16.
    # row q = y0*PW + x0 holds [xz[y0,x0,c], xz[y0+1,x0,c], xz[y0,x0+1,c],
    # xz[y0+1,x0+1,c]] for c in 0..63 -> 256 bf16 = 512B
    xz = np.zeros((PR + 2, PW, C), np.float32)
    xz[PADM:PADM + H, PADM:PADM + W] = xb.transpose(1, 2, 0)
    xzf = xz.reshape((PR + 2) * PW, C)
    tab = np.stack([xzf[0:NQ], xzf[PW:NQ + PW], xzf[1:NQ + 1],
                    xzf[PW + 1:NQ + PW + 1]], axis=2)  # [NQ, C, 4]
    ins["x2pad"] = tab.reshape(NQ, 4 * C).astype(bf16)
    # weight rearrangements
    wofft = np.zeros((C, K2 * 18), np.float32)
    for k in range(K2):
        wofft[:, 18 * k:18 * k + 18] = w_off[:, :, k // K, k % K].T
    ins["woffT"] = wofft.astype(bf16)
    wdeft = np.zeros((128, K2 * C), np.float32)
    for k in range(K2):
        blk = w_def[:, :, k // K, k % K].T  # [cin, cout]
        wdeft[0:64, C * k:C * k + C] = blk
        wdeft[64:128, C * k:C * k + C] = blk
    ins["wdefT"] = wdeft.astype(bf16)
    # base grids (pixel-major [128, 72] per tap), fold b_off and pad margin
    pixi = np.arange(HW, dtype=np.int64)
    ygrid = (pixi // W).astype(np.float32)
    xgrid = (pixi % W).astype(np.float32)
    ypm = ygrid.reshape(NB, 128).T    # [p, b] pixel-major
    xpm = xgrid.reshape(NB, 128).T
    bpy = np.zeros((128, K2 * NB), np.float32)
    bpx = np.zeros((128, K2 * NB), np.float32)
    for k in range(K2):
        ky, kx = k // K - 1, k % K - 1
        bpy[:, NB * k:NB * k + NB] = ypm + (ky + PADM + b_off[2 * k])
        bpx[:, NB * k:NB * k + NB] = xpm + (kx + PADM + b_off[2 * k + 1])
    ins["bpy"] = bpy
    ins["bpx"] = bpx
    ins["ident"] = np.eye(128, dtype=np.float32)
    ins["bnc"] = np.stack([gamma, beta], axis=1).astype(np.float32)
    return ins


def _get_nc():
    if "nc" not in _CACHE:
        nc = bacc.Bacc("TRN2", target_bir_lowering=False, debug=False,
                       num_devices=NCORES, num_swdge_queues=2)
        _build(nc)
        nc.compile()
        _CACHE["nc"] = nc
    return _CACHE["nc"]


def kernel(x, w_off, b_off, w_def, b_def, gamma, beta, trace=False, tmpdir=None):
    x = np.asarray(x, np.float32)
    w_off = np.asarray(w_off, np.float32)
    b_off = np.asarray(b_off, np.float32)
    w_def = np.asarray(w_def, np.float32)
    gamma = np.asarray(gamma, np.float32)
    beta = np.asarray(beta, np.float32)
    # b_def cancels exactly in training-mode BN; accepted but unused.
    nc = _get_nc()
    in_maps = [_prep_core(x[b], w_off, b_off, w_def, gamma, beta)
               for b in range(B)]
    res = bass_utils.run_bass_kernel_spmd(
        nc, in_maps, core_ids=list(range(NCORES)), trace=trace, tmpdir=tmpdir)
    out = np.stack([res.results[b]["out"].reshape(C, H, W) for b in range(B)])
    if trace:
        kernel.last_exec_time_ns = res.exec_time_ns
        kernel.last_results = res
    return out



# revision 17
# speedup vs baseline: 1.3455x; 1.3360x over previous
"""Deformable conv block (offset conv -> bilinear sample -> conv -> BN -> ReLU)
on 8 Trainium2 NeuronCores, data-parallel over batch.

Self-contained: hardcodes all shapes. kernel(**inputs) takes full inputs,
shards batch across 8 cores, runs one Bass/Tile SPMD program, returns the
full [8, 64, 96, 96] float32 output.

v2 (bf16 pipeline):
  - gather table rows hold all 4 bilinear corners channel-interleaved in
    bf16 (512B rows) -> one 512B descriptor per (pixel, tap), 42.5 MB/core
  - one merged dma_gather per (chunk, tap): 1536 idxs (amortizes the
    ~1us per-instruction gpsimd descriptor-gen cost)
  - bilinear combine = broadcast mult + grouped 4->1 tensor_reduce (DVE,
    bf16, corner values adjacent so DVE perf modes stay on)
  - pixel-major -> channel-major pair transpose on the XBAR
    (dma_start_transpose) instead of PE matmul transposes
  - int16 gather indices built with wrap/doubling DMAs instead of PE
    double-transposes
  - offset conv + deformable conv matmuls in bf16 (fp32 PSUM accumulate)
  - offsets/weights/index math stays fp32 on DVE
  - BN stats AllReduce across 8 cores, scale/bias fold, ReLU, un-permute out
"""
import os
from contextlib import ExitStack

import numpy as np
import ml_dtypes

import concourse.bass as bass
import concourse.tile as tile
from concourse import bacc, mybir, bass_utils

dt = mybir.dt
AOT = mybir.AluOpType
AFT = mybir.ActivationFunctionType

# problem shapes
B, C, H, W, K = 8, 64, 96, 96, 3
HW = H * W                # 9216
K2 = K * K                # 9
NCORES = 8
EPS = 1e-5

# padded sample-grid geometry: padded coord = image coord + PADM
PADM = 3                  # margin for floor(py) in [0, 100]
PW = W + 2 * PADM + 1     # 103 padded grid width
PR = H + 2 * PADM + 1     # 103 padded grid rows
NQ = PW * PR              # 10609 rows in the 4-corner gather table
QCLAMP = float(W + 2 * PADM - 2)  # 100: floor clamp ceiling

NB = HW // 128            # 72 pixel-major block columns
CHUNK = 8                 # block columns per main-loop chunk (1024 idx = 32KB
NCH = NB // CHUNK         # per DMA engine: the single_packet gather limit)
NPIX = CHUNK * 128        # 1024 pixels per chunk
MAGIC = 8388608.0         # 2^23

_CACHE = {}


def _build(nc, ncores=NCORES, use_collective=True):
    STOP = os.environ.get("KSTOP", "full")
    xpad = nc.dram_tensor("xpad", [C, 98 * 98], dt.bfloat16, kind="ExternalInput").ap()
    x2pad = nc.dram_tensor("x2pad", [NQ, 256], dt.bfloat16, kind="ExternalInput").ap()
    woffT = nc.dram_tensor("woffT", [C, K2 * 18], dt.bfloat16, kind="ExternalInput").ap()
    wdefT = nc.dram_tensor("wdefT", [128, K2 * C], dt.bfloat16, kind="ExternalInput").ap()
    bpy = nc.dram_tensor("bpy", [128, K2 * NB], dt.float32, kind="ExternalInput").ap()
    bpx = nc.dram_tensor("bpx", [128, K2 * NB], dt.float32, kind="ExternalInput").ap()
    ident = nc.dram_tensor("ident", [128, 128], dt.float32, kind="ExternalInput").ap()
    bnc = nc.dram_tensor("bnc", [C, 2], dt.float32, kind="ExternalInput").ap()
    out_d = nc.dram_tensor("out", [C, HW], dt.float32, kind="ExternalOutput").ap()

    with tile.TileContext(nc) as tc:
        with ExitStack() as ctx:
            cpool = ctx.enter_context(tc.tile_pool(name="const", bufs=1))
            ppool = ctx.enter_context(tc.tile_pool(name="persist", bufs=1))
            spool = ctx.enter_context(tc.tile_pool(name="small", bufs=3))
            gpool = ctx.enter_context(tc.tile_pool(name="gather", bufs=5))
            tpool = ctx.enter_context(tc.tile_pool(name="tmul", bufs=4))
            wpool = ctx.enter_context(tc.tile_pool(name="work", bufs=3))
            dpool = ctx.enter_context(tc.tile_pool(name="dram", bufs=1, space="DRAM"))
            ps_m = ctx.enter_context(tc.tile_pool(name="ps_m", bufs=2, space="PSUM"))
            ps_o = ctx.enter_context(tc.tile_pool(name="ps_o", bufs=2, space="PSUM"))
            ps_t = ctx.enter_context(tc.tile_pool(name="ps_t", bufs=2, space="PSUM"))

            # ---- load constants ----
            woffT_s = cpool.tile([C, K2 * 18], dt.bfloat16)
            nc.sync.dma_start(woffT_s[:], woffT)
            wdefT_s = cpool.tile([128, K2 * C], dt.bfloat16)
            nc.sync.dma_start(wdefT_s[:], wdefT)
            bpy_s = cpool.tile([128, K2 * NB], dt.float32)
            nc.sync.dma_start(bpy_s[:], bpy)
            bpx_s = cpool.tile([128, K2 * NB], dt.float32)
            nc.sync.dma_start(bpx_s[:], bpx)
            id_s = cpool.tile([128, 128], dt.float32)
            nc.sync.dma_start(id_s[:], ident)
            id_b = cpool.tile([128, 128], dt.bfloat16)
            nc.vector.tensor_copy(id_b[:], id_s[:])
            bnc_s = cpool.tile([C, 2], dt.float32)
            nc.sync.dma_start(bnc_s[:], bnc)

            # ---- 1+2. offset conv (streamed) + transpose to pixel-major ----
            # offT_s[p, 18*b + j] = off[j, 128*b + p]
            offT_s = ppool.tile([128, NB * 18], dt.float32)
            xpv = xpad.rearrange("c (h w) -> c h w", w=98)
            for cc in range(24):            # chunks of 4 rows = 384 px = 3 blocks
                xpc = wpool.tile([C, 6 * 98], dt.bfloat16, tag="xpc")
                nc.sync.dma_start(
                    xpc[:].rearrange("c (h w) -> c h w", w=98),
                    xpv[:, 4 * cc: 4 * cc + 6, :])
                xv = xpc[:].rearrange("c (h w) -> c h w", w=98)
                po = ps_m.tile([18, 384], dt.float32, tag="ps_misc")
                for k in range(K2):
                    ky, kx = k // K, k % K
                    rhs = xv[:, ky: ky + 4, kx: kx + 96]
                    nc.tensor.matmul(po[:], woffT_s[:, 18 * k: 18 * k + 18], rhs,
                                     start=(k == 0), stop=(k == K2 - 1))
                offc = wpool.tile([18, 384], dt.float32, tag="offc")
                nc.scalar.copy(offc[:], po[:])
                for cb in range(3):
                    pt = ps_m.tile([128, 18], dt.float32, tag="ps_misc")
                    nc.tensor.transpose(pt[:], offc[:, 128 * cb: 128 * cb + 128],
                                        id_s[0:18, 0:18])
                    c = 3 * cc + cb
                    nc.vector.tensor_copy(offT_s[:, 18 * c: 18 * c + 18], pt[:])
            offT_v = offT_s[:].rearrange("p (b j) -> p b j", j=18)
            if STOP == "off":
                nc.sync.dma_start(out_d[:, 0:NB * 18].rearrange("c (a b) -> c a b", a=2),
                                  offT_s[:].rearrange("p (a b) -> p a b", a=2)[0:64])
                return

            # ---- 3. per tap: bilinear weights w4 (bf16) + int16 gather idx ----
            # w4b[k][p, b, j]: j in (c00, c10, c01, c11) matching table row order
            w4b_taps = []
            idx_taps = []
            for k in range(K2):
                idx_k = ppool.tile([128, HW // 16], dt.int16, tag=f"idx{k}",
                                   name=f"idx_tap{k}")
                idx_taps.append(idx_k)
            for k in range(K2):
                py = spool.tile([128, NB], dt.float32, tag="py")
                nc.vector.tensor_tensor(py[:], offT_v[:, :, 2 * k],
                                        bpy_s[:, NB * k: NB * k + NB], AOT.add)
                px = spool.tile([128, NB], dt.float32, tag="px")
                nc.vector.tensor_tensor(px[:], offT_v[:, :, 2 * k + 1],
                                        bpx_s[:, NB * k: NB * k + NB], AOT.add)
                ry = spool.tile([128, NB], dt.float32, tag="ry")
                nc.vector.tensor_scalar(ry[:], py[:], MAGIC - 0.5, None, AOT.add)
                fy = spool.tile([128, NB], dt.float32, tag="fy")
                nc.vector.tensor_scalar(fy[:], ry[:], MAGIC, None, AOT.subtract)
                rx = spool.tile([128, NB], dt.float32, tag="rx")
                nc.vector.tensor_scalar(rx[:], px[:], MAGIC - 0.5, None, AOT.add)
                fx = spool.tile([128, NB], dt.float32, tag="fx")
                nc.vector.tensor_scalar(fx[:], rx[:], MAGIC, None, AOT.subtract)
                ly = spool.tile([128, NB], dt.float32, tag="ly")
                nc.vector.tensor_tensor(ly[:], py[:], fy[:], AOT.subtract)
                lx = spool.tile([128, NB], dt.float32, tag="lx")
                nc.vector.tensor_tensor(lx[:], px[:], fx[:], AOT.subtract)
                wy0 = spool.tile([128, NB], dt.float32, tag="wy0")
                nc.vector.tensor_scalar(wy0[:], ly[:], -1.0, 1.0, AOT.mult, AOT.add)
                wx0 = spool.tile([128, NB], dt.float32, tag="wx0")
                nc.vector.tensor_scalar(wx0[:], lx[:], -1.0, 1.0, AOT.mult, AOT.add)
                w4f = spool.tile([128, NB * 4], dt.float32, tag="w4f")
                w4fv = w4f[:].rearrange("p (b j) -> p b j", j=4)
                nc.vector.tensor_tensor(w4fv[:, :, 0], wy0[:], wx0[:], AOT.mult)
                nc.vector.tensor_tensor(w4fv[:, :, 1], ly[:], wx0[:], AOT.mult)
                nc.vector.tensor_tensor(w4fv[:, :, 2], wy0[:], lx[:], AOT.mult)
                nc.vector.tensor_tensor(w4fv[:, :, 3], ly[:], lx[:], AOT.mult)
                w4b = ppool.tile([128, NB * 4], dt.bfloat16, tag=f"w4b{k}")
                nc.vector.tensor_copy(w4b[:], w4f[:])
                w4b_taps.append(w4b)
                # gather row index q = qy * PW + qx (exact small integers)
                qy = spool.tile([128, NB], dt.float32, tag="qy")
                nc.vector.tensor_scalar(qy[:], fy[:], 0.0, QCLAMP, AOT.max, AOT.min)
                qx = spool.tile([128, NB], dt.float32, tag="qx")
                nc.vector.tensor_scalar(qx[:], fx[:], 0.0, QCLAMP, AOT.max, AOT.min)
                qf = spool.tile([128, NB], dt.float32, tag="qf")
                nc.vector.scalar_tensor_tensor(qf[:], qy[:], float(PW), qx[:],
                                               AOT.mult, AOT.add)
                # idx 16-row-wrap via PE double-transpose (T1 then 8x T2)
                t1p = ps_m.tile([NB, 128], dt.float32, tag="ps_misc")
                nc.tensor.transpose(t1p[:], qf[:], id_s[:])
                t1s = spool.tile([NB, 128], dt.float32, tag="t1s")
                nc.vector.tensor_copy(t1s[:], t1p[:])
                idx_k = idx_taps[k]
                stv = idx_k[0:16, :].rearrange("p (b r) -> p b r", r=8)
                for r in range(8):
                    t2p = ps_m.tile([16, NB], dt.float32, tag="ps_misc")
                    nc.tensor.transpose(t2p[:], t1s[:, 16 * r: 16 * r + 16],
                                        id_s[0:NB, 0:NB])
                    nc.any.tensor_copy(stv[:, :, r], t2p[:])
                # ---- 4. replicate idx rows 0..15 to all 8 16-row groups ----
                nc.sync.dma_start(idx_k[16:32, :], idx_k[0:16, :])
                nc.sync.dma_start(idx_k[32:64, :], idx_k[0:32, :])
                nc.sync.dma_start(idx_k[64:128, :], idx_k[0:64, :])

            if STOP == "idx":
                nc.sync.dma_start(
                    out_d[0:64, 0:HW // 16].bitcast(dt.int16)[:, 0:HW // 16],
                    idx_taps[0][0:64, :])
                return

            # ---- 5. main loop: gather -> weight+reduce -> xbarT -> conv ----
            x2win = bass.AP(x2pad.tensor, 0, [[256, NQ], [1, 256]])
            conv_s = ppool.tile([C, HW], dt.float32)
            sums = ppool.tile([C, 12], dt.float32)
            sqs = ppool.tile([C, 12], dt.float32)
            NCH_RUN = 1 if STOP in ("g1", "wsum", "tr") else NCH
            NIC = NPIX // 16          # idx cols per (chunk, tap): 64
            for ch in range(NCH_RUN):
                # 2 bank-aligned accumulation groups: even | odd blocks
                po = ps_o.tile([C, 1024], dt.float32, tag="ps_out")
                for k in range(K2):
                    g_t = gpool.tile([128, CHUNK * 256], dt.bfloat16, tag="g")
                    nc.gpsimd.dma_gather(
                        out_ap=g_t[:].rearrange("p (b e) -> p b e", e=256),
                        in_ap=x2win,
                        idxs_ap=idx_taps[k][:, NIC * ch: NIC * ch + NIC],
                        num_idxs=NPIX,
                        num_idxs_reg=NPIX,
                        elem_size=256,
                        elem_step=256,
                        queue_num=1,
                    )
                    if STOP == "g1":
                        nc.sync.dma_start(
                            out_d[0:64, 0:CHUNK * 128].bitcast(dt.bfloat16)[:, 0:CHUNK * 256],
                            g_t[0:64, :])
                        return
                    # bilinear combine: t = g * w4 (broadcast over channels),
                    # then reduce over the 4 corners (innermost)
                    gv4 = g_t[:].rearrange("p (b c j) -> p b c j", c=C, j=4)
                    w4v = w4b_taps[k][:].rearrange("p (b u j) -> p b u j", u=1, j=4)
                    wj = w4v[:, CHUNK * ch: CHUNK * ch + CHUNK]
                    a1, a2 = bass.broadcast_tensor_aps(gv4, wj)
                    t_t = tpool.tile([128, CHUNK * 256], dt.bfloat16, tag="t")
                    nc.vector.tensor_tensor(
                        t_t[:].rearrange("p (b c j) -> p b c j", c=C, j=4), a1, a2,
                        AOT.mult)
                    # 4-corner sum as two adds: pairs (2x-mode) then final
                    tv = t_t[:].rearrange("p (bc j) -> p bc j", j=4)
                    s2 = wpool.tile([128, CHUNK * C * 2], dt.bfloat16, tag="s2",
                                    bufs=6)
                    s2v = s2[:].rearrange("p (bc j) -> p bc j", j=2)
                    nc.vector.tensor_tensor(s2v, tv[:, :, 0:2], tv[:, :, 2:4],
                                            AOT.add)
                    s_t = wpool.tile([128, CHUNK * C], dt.bfloat16, tag="s",
                                     bufs=10)
                    nc.vector.tensor_tensor(s_t[:], s2v[:, :, 0], s2v[:, :, 1],
                                            AOT.add)
                    if STOP == "wsum":
                        if k == K2 - 1:
                            nc.sync.dma_start(
                                out_d[0:64, 0:CHUNK * 32].bitcast(dt.bfloat16)[:, 0:CHUNK * C],
                                s_t[0:64, :])
                            return
                        continue
                    # pair transpose -> channel-major sampled (PE transposes)
                    samp = wpool.tile([128, CHUNK * C], dt.bfloat16, tag="samp",
                                      bufs=10)
                    tp_p = ps_t.tile([128, CHUNK * C], dt.bfloat16, tag="ps_tr")
                    for j in range(CHUNK * C // 128):
                        nc.tensor.transpose(tp_p[:, 128 * j: 128 * j + 128],
                                            s_t[:, 128 * j: 128 * j + 128],
                                            id_b[:])
                    nc.scalar.copy(samp[:], tp_p[:])
                    if STOP == "tr":
                        if k == K2 - 1:
                            nc.sync.dma_start(
                                out_d[0:64, 0:CHUNK * 32].bitcast(dt.bfloat16)[:, 0:CHUNK * C],
                                samp[0:64, :])
                            return
                        continue
                    # conv matmuls: accumulate over taps, parity-major out cols
                    st, sp = (k == 0), (k == K2 - 1)
                    lhe = wdefT_s[0:64, C * k: C * k + C]
                    lho = wdefT_s[64:128, C * k: C * k + C]
                    nc.tensor.matmul(po[:, 0:512], lhe, samp[0:64, :],
                                     start=st, stop=sp)
                    nc.tensor.matmul(po[:, 512:1024], lho, samp[64:128, :],
                                     start=st, stop=sp)
                # copy conv chunk to SBUF + per-chunk sum / sumsq
                cview = conv_s[:, NPIX * ch: NPIX * ch + NPIX]
                nc.scalar.activation(cview, po[:], AFT.Copy,
                                     accum_out=sums[:, ch: ch + 1])
                scr = wpool.tile([C, NPIX], dt.float32, tag="scr", bufs=1)
                nc.scalar.activation(scr[:], cview, AFT.Square,
                                     accum_out=sqs[:, ch: ch + 1])

            if STOP == "loop":
                nc.sync.dma_start(out_d[:], conv_s[:])
                return

            # ---- 6. BN stats allreduce + normalize + relu + output ----
            st2 = ppool.tile([C, 2], dt.float32)
            nc.vector.tensor_reduce(st2[:, 0:1], sums[:, 0:NCH],
                                    mybir.AxisListType.X, AOT.add)
            nc.vector.tensor_reduce(st2[:, 1:2], sqs[:, 0:NCH],
                                    mybir.AxisListType.X, AOT.add)
            bi = dpool.tile([C, 2], dt.float32)
            bo = dpool.tile([C, 2], dt.float32)
            nc.sync.dma_start(bi[:], st2[:])
            if use_collective:
                nc.gpsimd.collective_compute(
                    "AllReduce", AOT.add,
                    replica_groups=[list(range(ncores))],
                    ins=[bi.opt()], outs=[bo.opt()])
            else:
                nc.sync.dma_start(bo[:], bi[:])
            ast = ppool.tile([C, 2], dt.float32)
            nc.sync.dma_start(ast[:], bo[:])

            inv_n = 1.0 / float(ncores * HW)
            mean = ppool.tile([C, 1], dt.float32)
            nc.vector.tensor_scalar(mean[:], ast[:, 0:1], inv_n, None, AOT.mult)
            msq = ppool.tile([C, 1], dt.float32)
            nc.vector.tensor_scalar(msq[:], ast[:, 1:2], inv_n, None, AOT.mult)
            m2 = ppool.tile([C, 1], dt.float32)
            nc.vector.tensor_tensor(m2[:], mean[:], mean[:], AOT.mult)
            var = ppool.tile([C, 1], dt.float32)
            nc.vector.tensor_tensor(var[:], msq[:], m2[:], AOT.subtract)
            vare = ppool.tile([C, 1], dt.float32)
            nc.vector.tensor_scalar(vare[:], var[:], EPS, None, AOT.add)
            sd = ppool.tile([C, 1], dt.float32)
            nc.scalar.activation(sd[:], vare[:], AFT.Sqrt)
            inv = ppool.tile([C, 1], dt.float32)
            nc.vector.reciprocal(inv[:], sd[:])
            scl = ppool.tile([C, 1], dt.float32)
            nc.vector.tensor_tensor(scl[:], bnc_s[:, 0:1], inv[:], AOT.mult)
            mt = ppool.tile([C, 1], dt.float32)
            nc.vector.tensor_tensor(mt[:], mean[:], scl[:], AOT.mult)
            bia = ppool.tile([C, 1], dt.float32)
            nc.vector.tensor_tensor(bia[:], bnc_s[:, 1:2], mt[:], AOT.subtract)

            ov = out_d.rearrange("c (n q) -> c n q", q=128)
            for ch in range(NCH):
                on = wpool.tile([C, NPIX], dt.float32, tag="on")
                nc.scalar.activation(on[:], conv_s[:, NPIX * ch: NPIX * ch + NPIX],
                                     AFT.Relu, bias=bia[:], scale=scl[:])
                onv = on[:].rearrange("c (n q) -> c n q", q=128)
                # even local blocks -> even global block slots, odd -> odd
                nc.sync.dma_start(ov[:, CHUNK * ch: CHUNK * ch + CHUNK: 2, :],
                                  onv[:, 0:CHUNK // 2, :])
                nc.sync.dma_start(ov[:, CHUNK * ch + 1: CHUNK * ch + CHUNK: 2, :],
                                  onv[:, CHUNK // 2:CHUNK, :])


def _prep_core(xb, w_off, b_off, w_def, gamma, beta):
    """Host-side input prep for one batch item. xb: [64, 96, 96] f32."""
    bf16 = ml_dtypes.bfloat16
    ins = {}
    # xpad: zero-pad by 1 for the 3x3 offset conv (bf16)
    xp = np.zeros((C, 98, 98), np.float32)
    xp[:, 1:97, 1:97] = xb
    ins["xpad"] = xp.reshape(C, 98 * 98).astype(bf16)
    # x2pad: 4-corner gather table, channel-interleaved bf16.
    # row q = y0*PW + x0 holds [xz[y0,x0,c], xz[y0+1,x0,c], xz[y0,x0+1,c],
    # xz[y0+1,x0+1,c]] for c in 0..63 -> 256 bf16 = 512B
    xz = np.zeros((PR + 2, PW, C), np.float32)
    xz[PADM:PADM + H, PADM:PADM + W] = xb.transpose(1, 2, 0)
    xzf = xz.reshape((PR + 2) * PW, C)
    tab = np.stack([xzf[0:NQ], xzf[PW:NQ + PW], xzf[1:NQ + 1],
                    xzf[PW + 1:NQ + PW + 1]], axis=2)  # [NQ, C, 4]
    ins["x2pad"] = tab.reshape(NQ, 4 * C).astype(bf16)
    # weight rearrangements
    wofft = np.zeros((C, K2 * 18), np.float32)
    for k in range(K2):
        wofft[:, 18 * k:18 * k + 18] = w_off[:, :, k // K, k % K].T
    ins["woffT"] = wofft.astype(bf16)
    wdeft = np.zeros((128, K2 * C), np.float32)
    for k in range(K2):
        blk = w_def[:, :, k // K, k % K].T  # [cin, cout]
        wdeft[0:64, C * k:C * k + C] = blk
        wdeft[64:128, C * k:C * k + C] = blk
    ins["wdefT"] = wdeft.astype(bf16)
    # base grids (pixel-major [128, 72] per tap), fold b_off and pad margin
    pixi = np.arange(HW, dtype=np.int64)
    ygrid = (pixi // W).astype(np.float32)
    xgrid = (pixi % W).astype(np.float32)
    ypm = ygrid.reshape(NB, 128).T    # [p, b] pixel-major
    xpm = xgrid.reshape(NB, 128).T
    bpy = np.zeros((128, K2 * NB), np.float32)
    bpx = np.zeros((128, K2 * NB), np.float32)
    for k in range(K2):
        ky, kx = k // K - 1, k % K - 1
        bpy[:, NB * k:NB * k + NB] = ypm + (ky + PADM + b_off[2 * k])
        bpx[:, NB * k:NB * k + NB] = xpm + (kx + PADM + b_off[2 * k + 1])
    ins["bpy"] = bpy
    ins["bpx"] = bpx
    ins["ident"] = np.eye(128, dtype=np.float32)
    ins["bnc"] = np.stack([gamma, beta], axis=1).astype(np.float32)
    return ins


def _get_nc():
    if "nc" not in _CACHE:
        nc = bacc.Bacc("TRN2", target_bir_lowering=False, debug=False,
                       num_devices=NCORES, num_swdge_queues=2)
        _build(nc)
        nc.compile()
        _CACHE["nc"] = nc
    return _CACHE["nc"]


def kernel(x, w_off, b_off, w_def, b_def, gamma, beta, trace=False, tmpdir=None):
    x = np.asarray(x, np.float32)
    w_off = np.asarray(w_off, np.float32)
    b_off = np.asarray(b_off, np.float32)
    w_def = np.asarray(w_def, np.float32)
    gamma = np.asarray(gamma, np.float32)
    beta = np.asarray(beta, np.float32)
    # b_def cancels exactly in training-mode BN; accepted but unused.
    nc = _get_nc()
    in_maps = [_prep_core(x[b], w_off, b_off, w_def, gamma, beta)
               for b in range(B)]
    res = bass_utils.run_bass_kernel_spmd(
        nc, in_maps, core_ids=list(range(NCORES)), trace=trace, tmpdir=tmpdir)
    out = np.stack([res.results[b]["out"].reshape(C, H, W) for b in range(B)])
    if trace:
        kernel.last_exec_time_ns = res.exec_time_ns
        kernel.last_results = res
    return out

